# revision 26
# baseline (speedup 1.0000x reference)
"""Trainium2 Bass kernel: segment-mean over contextual encodings.

Reference computation:
    emb  = concat([x[:, 257:769, :], broadcast(x[:, 0:1, :])], -1)   # [B, S, 2D]
    out  = scatter_mean(emb by segment_ids[:, 257:769]) -> [2048, 2D]

Sharding strategy: shard the OUTPUT segments across the 8 cores (256
segments each) so no collective is needed. Host-side prep partitions and
packs the token rows by owning (core, 128-seg bucket) into one contiguous
bf16 block per core in DMA-group-major order, so the device does pure
sequential HWDGE streaming — no indirect gather, no GpSimd/Pool engine.

Algebraic split: output columns [0:1024] are the segment-sum of x-window
rows (memory-bound, one-hot matmul accumulation); columns [1024:2048] are
the broadcast CLS row, whose segment-sum factorizes as per-(segment,batch)
counts @ x[:,0,:] — a tiny [32,128]T@[32,1024] matmul per bucket that
depends only on metadata + x0, so it runs and writes out EARLY, under the
input stream. Counts and reciprocals come from segment_ids alone and are
precomputed on the host.

Per core: stream ~18 chunks of 128 rows x 1024 bf16 (~4.5 MiB) with the
input DMAs alternating across the two HWDGE rings (Sync + ACT) for double
queue depth, build one-hot matrices on DVE (one op per DMA group),
accumulate segment sums with bf16 matmuls in PSUM, scale by host-provided
reciprocals, and write the 256-row output slice as bf16 (upcast to fp32
on the host; quantization error ~2^-9 ≪ the 2e-2 gate).
"""

import numpy as np

B = 32          # batch
TSEQ = 1024     # sequence length of x
D = 1024        # feature dim
SENT = 512
CTX = 256
NSEG = 2048
LO = 1 + CTX    # 257
HI = LO + SENT  # 769
NCORES = 8
SEGS_PER_CORE = NSEG // NCORES   # 256
P = 128
BUCKETS = SEGS_PER_CORE // P     # 2

LAST_RESULTS = None  # BassKernelResults of the most recent run (for test.py)


GSZ = 2  # chunks per input DMA (512KB): fine enough that the ~1us
         # DMA-completion latency pipelines instead of stalling consumers


def _balance_bins(seg_counts, caps):
    """Greedy-assign the 2048 segments to 16 (core,bucket) bins, exactly
    P segments per bin, minimizing load/cap. Returns [16, P] segment ids
    or None if a cap is violated."""
    nbins = len(caps)
    order = np.argsort(-seg_counts, kind="stable")
    loads = np.zeros(nbins)
    slots = [[] for _ in range(nbins)]
    for s in order:
        c = seg_counts[s]
        best, bestv = -1, None
        for i in range(nbins):
            if len(slots[i]) >= P:
                continue
            v = (loads[i] + c) / caps[i]
            if bestv is None or v < bestv:
                best, bestv = i, v
        slots[best].append(s)
        loads[best] += c
    if np.any(loads > np.asarray(caps)):
        return None
    return np.asarray(slots)


def _build_shards(xw32, x0_32, seg_flat):
    """Pack rows by owning (core, bucket) into DMA-group-major bf16 blocks
    plus local-segment labels, counts, reciprocals. Segments are
    rebalanced across bins so bucket1 fits 8 chunks (NCH=17 not 18)."""
    import ml_dtypes

    bf16 = ml_dtypes.bfloat16
    tok = np.nonzero(seg_flat >= 0)[0]
    tseg = seg_flat[tok]
    seg_counts = np.bincount(tseg, minlength=NSEG)

    for cbs in ((9, 8), (9, 9), (10, 10)):
        caps = [cbs[b] * P for _ in range(NCORES) for b in range(BUCKETS)]
        bins = _balance_bins(seg_counts, caps)
        if bins is not None:
            CBs = cbs
            break
    NCH = sum(CBs)
    cstart = (0, CBs[0])                      # chunk offset per bucket
    NG = -(-NCH // GSZ)
    EXTRA = 2 * NCH + 2 * BUCKETS             # segl + recip f32 as bf16

    # seg -> (bin, local slot)
    slot_of = np.empty(NSEG, np.int64)
    for i in range(len(bins)):
        slot_of[bins[i]] = np.arange(P)
    bin_of = np.empty(NSEG, np.int64)
    for i in range(len(bins)):
        bin_of[bins[i]] = i

    tb = bin_of[tseg]                          # token's bin
    tloc = slot_of[tseg]                       # token's local slot
    batch_id = tok // SENT

    xw16 = xw32.astype(bf16)
    widths = [min(GSZ, NCH - g * GSZ) * D for g in range(NG)]
    xs = [np.zeros((NCORES, P, w), bf16) for w in widths]
    segl = np.full((NCORES, P, NCH), -1.0, np.float32)
    cmT = np.zeros((NCORES, B, BUCKETS * P), np.float32)
    recip = np.ones((NCORES, P, BUCKETS), np.float32)
    for c in range(NCORES):
        for b in range(BUCKETS):
            m = tb == c * BUCKETS + b
            rows = tok[m]
            loc = tloc[m]
            bat = batch_id[m]
            n = rows.size
            CB = CBs[b]
            npad = CB * P
            data = np.zeros((npad, D), bf16)
            data[:n] = xw16[rows]
            lab = np.full(npad, -1.0, np.float32)
            lab[:n] = loc.astype(np.float32)
            chunks = data.reshape(CB, P, D)
            for k in range(CB):
                ci = cstart[b] + k
                gi, col = divmod(ci, GSZ)
                xs[gi][c, :, col * D:(col + 1) * D] = chunks[k]
            segl[c, :, cstart[b]:cstart[b] + CB] = lab.reshape(CB, P).T
            np.add.at(cmT[c], (bat, b * P + loc), 1.0)
            tot = np.bincount(loc, minlength=P)
            recip[c, :, b] = 1.0 / np.maximum(tot, 1.0)
    # flat per-core stream: group 0 carries the segl+recip metadata inline
    # (single dependency for the one-hot path, no separate small DMAs)
    meta = np.concatenate([segl.view(bf16), recip.view(bf16)],
                          axis=2)                       # [NC, P, EXTRA]
    blk0 = np.concatenate([xs[0], meta], axis=2)
    parts = [blk0.reshape(NCORES, -1)] + [g.reshape(NCORES, -1)
                                          for g in xs[1:]]
    flat = np.concatenate(parts, axis=1)
    return CBs, bins, flat, cmT.astype(bf16), x0_32.astype(bf16)


def _build_program(CBs):
    import concourse.bacc as bacc
    import concourse.tile as tile
    from concourse import mybir

    f32 = mybir.dt.float32
    bf16 = mybir.dt.bfloat16
    NCH = sum(CBs)
    cstart = (0, CBs[0])
    NG = -(-NCH // GSZ)
    widths = [min(GSZ, NCH - g * GSZ) * D for g in range(NG)]

    i32 = mybir.dt.int32
    EXTRA = 2 * NCH + 2 * BUCKETS
    NFLAT = P * (sum(widths) + EXTRA)
    nc = bacc.Bacc("TRN2", target_bir_lowering=False, debug=False,
                   num_devices=NCORES)
    xs_d = nc.dram_tensor("xs", [NFLAT], bf16, kind="ExternalInput")
    x0_d = nc.dram_tensor("x0", [B, D], bf16, kind="ExternalInput")
    cmT_d = nc.dram_tensor("cmT", [B, BUCKETS * P], bf16,
                           kind="ExternalInput")
    out_d = nc.dram_tensor("out", [SEGS_PER_CORE, 2 * D], bf16,
                           kind="ExternalOutput")

    with tile.TileContext(nc) as tc:
        with (
            tc.tile_pool(name="const", bufs=1) as constp,
            tc.tile_pool(name="data", bufs=NG) as datap,
            tc.tile_pool(name="oh", bufs=NG) as ohp,
            tc.tile_pool(name="outs", bufs=8) as outsp,
            tc.tile_pool(name="psum", bufs=2, space="PSUM") as psump,
        ):
            # cls deps go FIRST on the ACT ring (small; behind the bulk they
            # complete 15us late and stall the PE queue mid-stream)
            x0_sb = constp.tile([B, D], bf16)
            nc.scalar.dma_start(out=x0_sb[:], in_=x0_d.ap()[:])
            cmT_sb = constp.tile([B, BUCKETS * P], bf16)
            nc.scalar.dma_start(out=cmT_sb[:], in_=cmT_d.ap()[:])

            iota_i = constp.tile([P, P], i32)
            nc.gpsimd.iota(iota_i[:], pattern=[[1, P]], base=0,
                           channel_multiplier=0)
            iota_sb = constp.tile([P, P], f32)
            nc.vector.tensor_copy(out=iota_sb[:], in_=iota_i[:])

            # warm-up: the HW activity monitor duty-cycles engines to 4/8
            # until it sees a busy ~10us window; burn PE on dummy matmuls
            # from the barrier so the full-rate grant arrives before the
            # real matmul stream instead of ~10us into it
            dummy = constp.tile([P, 640], bf16)
            nc.gpsimd.memset(dummy[:], 0.0)
            dps = psump.tile([P, 512], f32, tag="cls")
            for _ in range(9):
                nc.tensor.matmul(out=dps[:], lhsT=dummy[:, 0:P],
                                 rhs=dummy[:, P:640], start=True, stop=True)

            # bulk stream: alternate groups across the two HWDGE rings;
            # group 0 carries segl+recip inline (bitcast f32 views below)
            gdata = []
            off = 0
            for gi in range(NG):
                w = widths[gi] + (EXTRA if gi == 0 else 0)
                dt_g = datap.tile([P, w], bf16, tag="data", name=f"g{gi}")
                eng = nc.sync if gi % 2 == 0 else nc.scalar
                eng.dma_start(
                    out=dt_g[:],
                    in_=xs_d.ap()[off:off + P * w]
                        .rearrange("(p c) -> p c", p=P))
                off += P * w
                gdata.append(dt_g)
            segl_sb = gdata[0][:, widths[0]:widths[0] + 2 * NCH].bitcast(f32)
            recip_sb = gdata[0][:, widths[0] + 2 * NCH:
                                widths[0] + EXTRA].bitcast(f32)

            # one-hot matrices, one DVE op per group (only need segl+iota)
            ohg = []
            for gi in range(NG):
                n = widths[gi] // D
                oh_all = ohp.tile([P, n * P], bf16, tag="oh")
                nc.vector.tensor_tensor(
                    out=oh_all[:].rearrange("p (g q) -> p g q", g=n),
                    in0=iota_sb[:].unsqueeze(1).to_broadcast([P, n, P]),
                    in1=segl_sb[:, gi * GSZ:gi * GSZ + n].unsqueeze(2)
                        .to_broadcast([P, n, P]),
                    op=mybir.AluOpType.is_equal)
                ohg.append(oh_all)

            out_ring = [nc.sync, nc.scalar]

            # cls half depends only on metadata + x0: compute + write early,
            # hidden under the input stream
            for b in range(BUCKETS):
                cls_ps = psump.tile([P, D], f32, tag="cls")
                for j in range(2):
                    nc.tensor.matmul(
                        out=cls_ps[:, j * 512:(j + 1) * 512],
                        lhsT=cmT_sb[:, b * P:(b + 1) * P],
                        rhs=x0_sb[:, j * 512:(j + 1) * 512],
                        start=True, stop=True)
                for j in range(2):
                    o2 = outsp.tile([P, 512], bf16, tag="o")
                    nc.scalar.activation(
                        out=o2[:], in_=cls_ps[:, j * 512:(j + 1) * 512],
                        func=mybir.ActivationFunctionType.Copy,
                        scale=recip_sb[:, b:b + 1])
                    out_ring[j].dma_start(
                        out=out_d.ap()[b * P:(b + 1) * P,
                                       D + j * 512:D + (j + 1) * 512],
                        in_=o2[:])

            acc = [psump.tile([P, D], f32, tag="acc", name=f"acc{i}")
                   for i in range(BUCKETS)]

            for ci in range(NCH):
                b = 0 if ci < CBs[0] else 1
                first = ci == cstart[b]
                last = ci == cstart[b] + CBs[b] - 1
                gi, col = divmod(ci, GSZ)

                for j in range(2):
                    nc.tensor.matmul(
                        out=acc[b][:, j * 512:(j + 1) * 512],
                        lhsT=ohg[gi][:, col * P:(col + 1) * P],
                        rhs=gdata[gi][:, col * D + j * 512:
                                      col * D + (j + 1) * 512],
                        start=first, stop=last)

                if last:
                    # window epilogue, halves in parallel: j0 on DVE,
                    # j1 on ACT, so the two scales don't serialize
                    o1 = outsp.tile([P, 512], bf16, tag="o")
                    nc.vector.tensor_scalar_mul(
                        out=o1[:], in0=acc[b][:, 0:512],
                        scalar1=recip_sb[:, b:b + 1])
                    nc.sync.dma_start(
                        out=out_d.ap()[b * P:(b + 1) * P, 0:512],
                        in_=o1[:])
                    o1b = outsp.tile([P, 512], bf16, tag="o")
                    nc.scalar.activation(
                        out=o1b[:], in_=acc[b][:, 512:1024],
                        func=mybir.ActivationFunctionType.Copy,
                        scale=recip_sb[:, b:b + 1])
                    nc.scalar.dma_start(
                        out=out_d.ap()[b * P:(b + 1) * P, 512:1024],
                        in_=o1b[:])

    nc.compile()
    return nc


def kernel(x, segment_ids):
    global LAST_RESULTS
    from concourse.bass_utils import run_bass_kernel_spmd

    x = np.asarray(x, dtype=np.float32)
    seg_all = np.asarray(segment_ids).astype(np.int64)
    assert x.shape == (B, TSEQ, D), x.shape
    assert seg_all.shape == (B, TSEQ), seg_all.shape

    xw = np.ascontiguousarray(x[:, LO:HI, :].reshape(B * SENT, D))
    x0 = np.ascontiguousarray(x[:, 0, :])
    seg_flat = seg_all[:, LO:HI].reshape(-1)

    CBs, bins, flat, cmT, x0_16 = _build_shards(xw, x0, seg_flat)
    nc = _build_program(CBs)

    in_maps = [
        {"xs": flat[c], "x0": x0_16, "cmT": cmT[c]}
        for c in range(NCORES)
    ]
    last_err = None
    for _attempt in range(3):
        try:
            res = run_bass_kernel_spmd(nc, in_maps, list(range(NCORES)))
            break
        except Exception as e:  # transient NRT device errors happen; retry
            last_err = e
    else:
        raise last_err
    LAST_RESULTS = res
    out = np.empty((NSEG, 2 * D), np.float32)
    for c in range(NCORES):
        rows = res.results[c]["out"].astype(np.float32)   # [256, 2D]
        for b in range(BUCKETS):
            out[bins[c * BUCKETS + b]] = rows[b * P:(b + 1) * P]
    return out
